# revision 10
# baseline (speedup 1.0000x reference)
"""Trainium2 Bass kernel for nn_DynamicShortConvolution.

Reference computation (per token t, channel d):
    h    = silu(x @ w1)                       # [T, H]
    flat = h @ w2 + b2                        # [T, D*W]
    k    = flat.reshape(T, D, W)
    out[t, d] = silu(sum_w k[t, d, w] * x[t - (W-1) + w, d])

Sharding: 8 cores, each one (batch, half-of-T) shard of 2048 tokens plus a
3-token left halo.  All per-core tensors are laid out transposed ([D, T] with
channels on SBUF partitions) so the conv's token shift is a free-dim offset.

v2 design (epilogue-balanced):
  - mm2 writes per-tap psum tiles [128,1024] fp32 (2 banks, 3-deep rotation).
  - Evacuation split: ACT copies taps {0,2} to bf16, DVE does taps {1,3} as
    fused scalar_tensor_tensor (+bias, *shifted-x) at 1x.  ACT-taps' bias and
    x-multiply happen on DVE as bf16 STT at 2x mode (even-column x slices are
    4B-aligned; odd columns ride the 1x psum STTs).
  - Two adds per qunit alternate DVE/GPSIMD; final silu on ACT, deferred by
    one qunit so ACT's queue never blocks on the add chain.
  - x is DMA'd in 512-token chunks so mm1 starts ~6us in; w2 arrives w-major
    between x chunks, matching mm2's tap order.
"""

import numpy as np

USE_CCE_ACC = True

# Problem constants (hardcoded per harness contract).
B, T, D, H, W = 4, 4096, 2048, 256, 4
HALO = W - 1
N_CORES = 8
TOK = (B * T) // N_CORES  # tokens per core = 2048


def _build_nc(tok, d, h, xstride):
    import concourse.bass as bass
    import concourse.bacc as bacc
    import concourse.mybir as mybir
    import concourse.tile as tile

    f32 = mybir.dt.float32
    bf16 = mybir.dt.bfloat16
    AF = mybir.ActivationFunctionType
    ALU = mybir.AluOpType

    n_dt = d // 128        # 16 d tiles
    n_hc = h // 128        # 2 h tiles
    n_ch = tok // 512      # 4 mm1 chunks
    n_pair = tok // 1024   # 2 token pairs (epilogue granularity)

    nc = bacc.Bacc()

    # DRAM inputs (host-prepared, partition-major layouts)
    xh = nc.declare_dram_parameter("xh", [128, n_dt, HALO], bf16, isOutput=False)
    xc = nc.declare_dram_parameter("xc", [n_ch, 128, n_dt, 512], bf16, isOutput=False)
    w1p = nc.declare_dram_parameter("w1r", [128, n_dt * h], bf16, isOutput=False)
    # w2r[k=w*n_hc+hc, p=hl, dt*128+dl] = w2[hc*128+p, (dt*128+dl)*W + w]
    w2p = nc.declare_dram_parameter("w2r", [W * n_hc, 128, d], bf16, isOutput=False)
    b2p = nc.declare_dram_parameter("b2r", [128, n_dt * W], f32, isOutput=False)
    outT = nc.declare_dram_parameter("outT", [n_dt, 128, tok], bf16, isOutput=True)

    with tile.TileContext(nc) as tc:
        with (
            tc.tile_pool(name="resident", bufs=1) as rpool,
            tc.tile_pool(name="work", bufs=3) as wpool,
            tc.tile_pool(name="ps2", bufs=3, space="PSUM") as pp2,
            tc.tile_pool(name="ps1", bufs=2, space="PSUM") as pp1,
        ):
            # ---- resident tiles ----
            xT = rpool.tile([128, n_dt, xstride], bf16, tag="x")
            w1_sb = rpool.tile([128, n_dt * h], bf16, tag="w1")
            w2_sb = rpool.tile([128, W * n_hc * d], bf16, tag="w2")
            b2_sb = rpool.tile([128, n_dt * W], f32, tag="b2")
            hT = rpool.tile([128, n_hc * tok], bf16, tag="hT")
            wrm = rpool.tile([128, 8], f32, tag="wrm")

            # ---- ACT table warm (silu set) during the DMA wait ----
            nc.vector.memset(wrm[:], 0.0)
            nc.scalar.activation(wrm[:, 4:5], wrm[:, 0:1], AF.Silu)
            # ---- PE HAM warmup: ~5us of dummy matmuls during the DMA wait ----
            dmy = rpool.tile([128, 640], bf16, tag="dmy")
            nc.vector.memset(dmy[:], 0.0)
            wp = pp1.tile([128, 512], f32, tag="hps")
            for _ in range(12):
                nc.tensor.matmul(wp[:], dmy[:, :128], dmy[:, 128:640],
                                 start=True, stop=True)

            # ---- input DMAs, ordered to feed the pipeline ----
            nc.sync.dma_start(w1_sb[:], w1p[:])
            nc.sync.dma_start(b2_sb[:], b2p[:])
            nc.sync.dma_start(xT[:, :, 0:HALO], xh[:])
            nc.sync.dma_start(xT[:, :, HALO + 0 * 512: HALO + 1 * 512], xc[0])
            nc.sync.dma_start(xT[:, :, HALO + 1 * 512: HALO + 2 * 512], xc[1])
            for k in range(W * n_hc):
                nc.sync.dma_start(w2_sb[:, k * d:(k + 1) * d], w2p[k])
            nc.sync.dma_start(xT[:, :, HALO + 2 * 512: HALO + 3 * 512], xc[2])
            nc.sync.dma_start(xT[:, :, HALO + 3 * 512: HALO + 4 * 512], xc[3])

            def xs(dt, col, n):
                return xT[:, dt, col: col + n]

            # deferred-silu state: (acc_tile, dt, j0) pending final silu+store
            pending = []

            def flush_pending(depth=0):
                while len(pending) > depth:
                    acc, fdt, fj0 = pending.pop(0)
                    ot = wpool.tile([128, 1024], bf16, tag="ot", bufs=4)
                    nc.scalar.activation(ot[:], acc, AF.Silu)
                    nc.scalar.dma_start(outT[fdt][:, fj0: fj0 + 1024], ot[:])

            def emit_mm1(c, hc):
                hp = pp1.tile([128, 512], f32, tag="hps")
                for dt in range(n_dt):
                    nc.tensor.matmul(
                        hp[:],
                        w1_sb[:, dt * h + hc * 128: dt * h + hc * 128 + 128],
                        xs(dt, HALO + c * 512, 512),
                        start=(dt == 0), stop=(dt == n_dt - 1),
                    )
                nc.scalar.activation(
                    hT[:, hc * tok + c * 512: hc * tok + (c + 1) * 512],
                    hp[:], AF.Silu)

            def emit_qunit(pair, dt, carry, dve_add):
                """One qunit: 4 tap matmul groups; evac interleaved so every
                cross-engine wait is exact (id0->e0->m1->id2->e2->m3); the
                carried s-add/acc of the previous qunit fills DVE/GPS idle."""
                j0 = pair * 1024
                kws = []
                for w in range(W):
                    kw = pp2.tile([128, 1024], f32, tag="kw")
                    for half in range(2):
                        for hc in range(n_hc):
                            nc.tensor.matmul(
                                kw[:, half * 512:(half + 1) * 512],
                                w2_sb[:, (w * n_hc + hc) * d + dt * 128:
                                      (w * n_hc + hc) * d + dt * 128 + 128],
                                hT[:, hc * tok + j0 + half * 512:
                                   hc * tok + j0 + (half + 1) * 512],
                                start=(hc == 0), stop=(hc == n_hc - 1),
                            )
                    kws.append(kw)
                # previous qunit's adds first: inputs are old, fills the
                # window while this qunit's taps stream through the PE
                if carry is not None:
                    ce, cm, cdt, cj0 = carry
                    s = wpool.tile([128, 2048], bf16, tag="s", bufs=5)
                    if dve_add:
                        nc.vector.tensor_add(s[:], ce[:], cm[:])
                    else:
                        nc.gpsimd.tensor_add(s[:], ce[:], cm[:])
                    if USE_CCE_ACC:
                        nc.gpsimd.dma_start(s[:, :1024], s[:, 1024:],
                                            accum_op=ALU.add)
                        acc = s[:, :1024]
                    else:
                        acc = wpool.tile([128, 1024], bf16, tag="acc", bufs=4)
                        nc.vector.tensor_add(acc[:], s[:, :1024], s[:, 1024:])
                    pending.append((acc, cdt, cj0))

                bias = [b2_sb[:, dt * W + w: dt * W + w + 1] for w in range(W)]
                kb = wpool.tile([128, 2048], bf16, tag="kb", bufs=4)
                m = wpool.tile([128, 2048], bf16, tag="m", bufs=4)
                e = wpool.tile([128, 2048], bf16, tag="e", bufs=4)
                # tap-ready order, e right after its id for an exact ACT wait
                nc.scalar.activation(kb[:, :1024], kws[0][:], AF.Identity,
                                     bias=bias[0])
                nc.vector.tensor_mul(e[:, :1024], kb[:, :1024],
                                     xs(dt, j0 + 0, 1024))
                nc.vector.scalar_tensor_tensor(
                    m[:, :1024], kws[1][:], bias[1], xs(dt, j0 + 1, 1024),
                    op0=ALU.add, op1=ALU.mult)
                nc.scalar.activation(kb[:, 1024:], kws[2][:], AF.Identity,
                                     bias=bias[2])
                nc.vector.tensor_mul(e[:, 1024:], kb[:, 1024:],
                                     xs(dt, j0 + 2, 1024))
                nc.vector.scalar_tensor_tensor(
                    m[:, 1024:], kws[3][:], bias[3], xs(dt, j0 + 3, 1024),
                    op0=ALU.add, op1=ALU.mult)
                return (e, m, dt, j0)

            # Schedule: pair0 mm1 upfront; pair1 mm1 groups interleaved
            # after every 4th pair0 qunit; e/add work deferred one qunit.
            # Emission order inside an iteration is back(q-1) -> mm1 group ->
            # front(q) -> silu flush: the dependency tracker's engine-counter
            # waits are captured at emission, so deferred ops must be emitted
            # before any newer work on the engines they wait on.
            qunits = [(0, i) for i in range(n_dt)] + [(1, i) for i in range(n_dt)]
            mm1_at = {0: [(c, hc) for c in (0, 1) for hc in range(n_hc)]}
            mm1_p1 = [(c, hc) for c in (2, 3) for hc in range(n_hc)]
            for i in range(4):
                mm1_at[4 * i + 4] = [mm1_p1[i]]

            carry = None
            for qidx, (pair, dt) in enumerate(qunits):
                for c, hc in mm1_at.get(qidx, []):
                    emit_mm1(c, hc)
                carry = emit_qunit(pair, dt, carry, dve_add=(qidx % 3 == 1))
                flush_pending(depth=2)
            ce, cm, cdt, cj0 = carry
            s = wpool.tile([128, 2048], bf16, tag="s", bufs=5)
            nc.vector.tensor_add(s[:], ce[:], cm[:])
            if USE_CCE_ACC:
                nc.gpsimd.dma_start(s[:, :1024], s[:, 1024:], accum_op=ALU.add)
                pending.append((s[:, :1024], cdt, cj0))
            else:
                acc = wpool.tile([128, 1024], bf16, tag="acc", bufs=4)
                nc.vector.tensor_add(acc[:], s[:, :1024], s[:, 1024:])
                pending.append((acc, cdt, cj0))
            flush_pending()
    nc.compile()
    return nc


def _prep_shards(x, w1, w2, b2, tok, d, h, xstride):
    import ml_dtypes
    bf16 = ml_dtypes.bfloat16

    n_dt = d // 128
    n_hc = h // 128
    n_ch = tok // 512
    b, t, _ = x.shape
    shards_per_batch = (b * t // tok) // b

    w1r = np.ascontiguousarray(
        w1.reshape(n_dt, 128, h).transpose(1, 0, 2).reshape(128, n_dt * h)
    ).astype(bf16)
    # w2r[w*n_hc+hc, hl, d]
    w2r = np.ascontiguousarray(
        w2.reshape(n_hc, 128, d, W).transpose(3, 0, 1, 2).reshape(W * n_hc, 128, d)
    ).astype(bf16)
    b2r = np.ascontiguousarray(
        b2.reshape(n_dt, 128, W).transpose(1, 0, 2).reshape(128, n_dt * W)
    ).astype(np.float32)

    in_maps = []
    for core in range(N_CORES):
        bi, half = divmod(core, shards_per_batch)
        t0 = half * tok
        xp = np.zeros((tok + HALO, d), np.float32)
        lo = max(t0 - HALO, 0)
        xp[HALO - (t0 - lo):] = x[bi, lo: t0 + tok]
        # [128, n_dt, tok+HALO] partition-major
        xt = np.ascontiguousarray(
            xp.T.reshape(n_dt, 128, tok + HALO).transpose(1, 0, 2)).astype(bf16)
        xh = np.ascontiguousarray(xt[:, :, 0:HALO])
        xc = np.ascontiguousarray(
            np.stack([xt[:, :, HALO + c * 512: HALO + (c + 1) * 512]
                      for c in range(n_ch)], axis=0))
        in_maps.append({
            "xh": xh, "xc": xc, "w1r": w1r, "w2r": w2r, "b2r": b2r})
    return in_maps


_NC_CACHE = {}


def kernel(x, w1, w2, b2, trace=False):
    from concourse.bass_utils import run_bass_kernel_spmd

    tok, d, h = TOK, D, H
    xstride = tok + HALO + 1  # even stride keeps per-dtile 4B alignment
    key = (tok, d, h)
    if key not in _NC_CACHE:
        _NC_CACHE[key] = _build_nc(tok, d, h, xstride)
    nc = _NC_CACHE[key]

    in_maps = _prep_shards(
        np.asarray(x, np.float32), np.asarray(w1, np.float32),
        np.asarray(w2, np.float32), np.asarray(b2, np.float32),
        tok, d, h, xstride)

    res = run_bass_kernel_spmd(nc, in_maps, core_ids=list(range(N_CORES)),
                               trace=trace)
    kernel.last_result = res

    shards_per_batch = (B * T // tok) // B
    out = np.empty((B, T, D), np.float32)
    for core in range(N_CORES):
        bi, half = divmod(core, shards_per_batch)
        oT = res.results[core]["outT"]  # [n_dt, 128, tok]
        out[bi, half * tok:(half + 1) * tok] = (
            oT.reshape(d, tok).T.astype(np.float32))
    return out
